# revision 5
# baseline (speedup 1.0000x reference)
"""Beam search (nn_BeamSearch) on Trainium2 — 8 NeuronCores, pure data parallel.

Input : logits float32 [8, 512, 32000]  (batch, length, vocab)
Output: (tokens int32 [8, 512, 3], scores float32 [8, 3])  — same as reference.

Sharding: batch dim across the 8 cores (one batch element per core, no
cross-core communication).

Device kernel (per core, logits shard [512, 32000]):
  For each 128-row group, stream four [128, 8000] slabs from DRAM and compute
    * per-125-element-chunk maxima (segmented DVE reduce)      -> cm [128, 256]
    * fused exp + accumulate per slab on the scalar engine     -> softmax sums
      (bias = -slab_max, so each slab's sum is sum(exp(x - slab_max)))
  Then per row: top-8 chunks by chunk max (max8 + max_index on cm), gather
  those 8 chunks (1000 elements) with an indirect DMA, and take max8 +
  max_index of the gathered window -> exact top-8 logit values + positions.
  The true top-k elements of a row always live in chunks whose maxima are
  among the top-k chunk maxima, so the window provably contains the row's
  top-8 elements.

Host: rebuild softmax log-probs for the top-5 tokens per (batch, step)
  (log p = (x - row_max) - log Z, with Z assembled from the per-slab sums)
  and run the reference's beam recurrence in float32 over the [8, 512, 5]
  survivors (the heavy 131M-element reduction work all happened on-device).
"""

import os
import numpy as np

B, L, V = 8, 512, 32000
P = 128                 # SBUF partitions (rows per group)
NGROUP = L // P         # 4 row groups per core
SLAB = 8000             # vocab elements per DMA/compute slab
NSLAB = V // SLAB       # 4
CHUNK = 125             # hierarchical top-k chunk size
NCHUNK = V // CHUNK     # 256 chunks per row
CPS = SLAB // CHUNK     # 64 chunks per slab
K8 = 8
BEAM_WIDTH = 3
TOP_TOKENS = 5
EPS = np.float32(2.220446049250313e-16)

_CACHE = {}


def _build_program():
    import concourse.bacc as bacc
    import concourse.tile as tile
    from concourse import bass, mybir

    nc = bacc.Bacc("TRN2", target_bir_lowering=False, debug=False, num_devices=B)

    f32 = mybir.dt.float32
    u32 = mybir.dt.uint32

    lg = nc.dram_tensor("logits", [L, V], f32, kind="ExternalInput")
    o_vals = nc.dram_tensor("o_vals", [L, K8], f32, kind="ExternalOutput")
    o_pos = nc.dram_tensor("o_pos", [L, K8], u32, kind="ExternalOutput")
    o_chunk = nc.dram_tensor("o_chunk", [L, K8], u32, kind="ExternalOutput")
    o_sums = nc.dram_tensor("o_sums", [L, NSLAB], f32, kind="ExternalOutput")
    o_negmax = nc.dram_tensor("o_negmax", [L, NSLAB], f32, kind="ExternalOutput")

    lg_ap = lg.ap()
    # [V-flattened] chunk table view for the indirect gather: row-chunk c of
    # row r is 125 contiguous floats at chunk-id r*256 + c.
    lg_chunks = lg.ap().rearrange("a (c k) -> (a c) k", k=CHUNK)

    with tile.TileContext(nc) as tc:
        with (
            tc.tile_pool(name="slabs", bufs=4) as slab_pool,
            tc.tile_pool(name="trash", bufs=1) as trash_pool,
            tc.tile_pool(name="stats", bufs=2) as stats_pool,
            tc.tile_pool(name="tiny", bufs=2) as tiny_pool,
            tc.tile_pool(name="win", bufs=2) as win_pool,
        ):
            trash = trash_pool.tile([P, SLAB], f32, tag="trash")
            for g in range(NGROUP):
                rows = slice(g * P, (g + 1) * P)
                cm = stats_pool.tile([P, NCHUNK], f32, tag="cm")
                negmax = stats_pool.tile([P, NSLAB], f32, tag="negmax")
                sums = stats_pool.tile([P, NSLAB], f32, tag="sums")
                for s in range(NSLAB):
                    slab = slab_pool.tile([P, SLAB], f32, tag="slab")
                    nc.sync.dma_start(
                        out=slab[:],
                        in_=lg_ap[rows, s * SLAB : (s + 1) * SLAB],
                    )
                    # per-chunk maxima for this slab
                    nc.vector.tensor_reduce(
                        out=cm[:, s * CPS : (s + 1) * CPS],
                        in_=slab[:].rearrange("p (c k) -> p c k", k=CHUNK),
                        axis=mybir.AxisListType.X,
                        op=mybir.AluOpType.max,
                    )
                    # slab max (negated, for the activation bias)
                    nc.vector.tensor_reduce(
                        out=negmax[:, s : s + 1],
                        in_=cm[:, s * CPS : (s + 1) * CPS],
                        axis=mybir.AxisListType.X,
                        op=mybir.AluOpType.max,
                        negate=True,
                    )
                    # sum(exp(x - slab_max)) fused on the scalar engine
                    nc.scalar.activation(
                        out=trash[:],
                        in_=slab[:],
                        func=mybir.ActivationFunctionType.Exp,
                        bias=negmax[:, s : s + 1],
                        scale=1.0,
                        accum_out=sums[:, s : s + 1],
                    )

                # top-8 chunks per row
                top8v = tiny_pool.tile([P, K8], f32, tag="top8v")
                top8c = tiny_pool.tile([P, K8], u32, tag="top8c")
                nc.vector.max(out=top8v[:], in_=cm[:])
                nc.vector.max_index(out=top8c[:], in_max=top8v[:], in_values=cm[:])

                # global chunk ids: (g*128 + p) * 256 + chunk
                gidx = tiny_pool.tile([P, K8], u32, tag="gidx")
                nc.gpsimd.iota(
                    out=gidx[:],
                    pattern=[[0, K8]],
                    base=g * P * NCHUNK,
                    channel_multiplier=NCHUNK,
                )
                nc.vector.tensor_tensor(
                    out=gidx[:], in0=gidx[:], in1=top8c[:], op=mybir.AluOpType.add
                )

                # gather the 8 winning chunks (125 elements each) per row.
                # NB: hardware indirect DMA honors one offset per partition,
                # so issue one gather per window slot.
                win = win_pool.tile([P, K8 * CHUNK], f32, tag="win")
                for j in range(K8):
                    nc.gpsimd.indirect_dma_start(
                        out=win[:, j * CHUNK : (j + 1) * CHUNK],
                        out_offset=None,
                        in_=lg_chunks,
                        in_offset=bass.IndirectOffsetOnAxis(
                            ap=gidx[:, j : j + 1], axis=0
                        ),
                    )

                # exact top-8 of the window = exact top-8 of the row
                wvals = tiny_pool.tile([P, K8], f32, tag="wvals")
                wpos = tiny_pool.tile([P, K8], u32, tag="wpos")
                nc.vector.max(out=wvals[:], in_=win[:])
                nc.vector.max_index(out=wpos[:], in_max=wvals[:], in_values=win[:])

                nc.sync.dma_start(out=o_vals.ap()[rows], in_=wvals[:])
                nc.sync.dma_start(out=o_pos.ap()[rows], in_=wpos[:])
                nc.sync.dma_start(out=o_chunk.ap()[rows], in_=top8c[:])
                nc.sync.dma_start(out=o_sums.ap()[rows], in_=sums[:])
                nc.sync.dma_start(out=o_negmax.ap()[rows], in_=negmax[:])
    nc.compile()
    return nc


def _get_program():
    if "nc" not in _CACHE:
        _CACHE["nc"] = _build_program()
    return _CACHE["nc"]


def _run_device(shards, trace=False):
    """shards: list of 8 [512, 32000] f32 arrays. Returns (per-core outputs,
    exec_time_ns or None)."""
    from concourse.bass_utils import run_bass_kernel_spmd

    nc = _get_program()
    in_maps = [{"logits": np.ascontiguousarray(s)} for s in shards]
    res = run_bass_kernel_spmd(nc, in_maps, core_ids=list(range(len(shards))), trace=trace)
    return res.results, res.exec_time_ns


def _device_outputs_numpy(shard):
    """Bit-faithful numpy emulation of the device kernel for one core.
    Used for validation / fallback (env BEAM_NO_HW=1)."""
    x = shard  # [L, V] f32
    cm = x.reshape(L, NCHUNK, CHUNK).max(axis=2)  # chunk maxima
    slab_max = x.reshape(L, NSLAB, SLAB).max(axis=2)
    negmax = (-slab_max).astype(np.float32)
    ex = np.exp(
        x.reshape(L, NSLAB, SLAB).astype(np.float32) + negmax[:, :, None]
    ).astype(np.float32)
    sums = ex.sum(axis=2, dtype=np.float32)
    # top-8 chunks (by max, ties -> lowest chunk id, descending)
    ordc = np.lexsort((np.arange(NCHUNK)[None, :].repeat(L, 0), -cm), axis=1)[:, :K8]
    top8c = ordc.astype(np.uint32)
    win = np.take_along_axis(
        x.reshape(L, NCHUNK, CHUNK), ordc[:, :, None], axis=1
    ).reshape(L, K8 * CHUNK)
    # device max_index returns positions of the 8 largest values in
    # descending-value order (ties -> successive lowest positions)
    wsort = np.lexsort((np.arange(K8 * CHUNK)[None, :].repeat(L, 0), -win), axis=1)[
        :, :K8
    ]
    wpos = wsort.astype(np.uint32)
    wvals = np.take_along_axis(win, wsort, axis=1).astype(np.float32)
    return {
        "o_vals": wvals,
        "o_pos": wpos,
        "o_chunk": top8c,
        "o_sums": sums,
        "o_negmax": negmax,
    }


def _postprocess(core_outs):
    """core_outs: list of 8 dicts with o_vals/o_pos/o_chunk/o_sums/o_negmax.
    Returns (tokens [8, 512, 3] int32, scores [8, 3] f32), exactly emulating
    the reference's float32 beam recurrence."""
    top_idx = np.empty((B, L, TOP_TOKENS), np.int64)
    top_logp = np.empty((B, L, TOP_TOKENS), np.float32)

    for b, o in enumerate(core_outs):
        vals = np.asarray(o["o_vals"], np.float32)        # [L, 8]
        pos = np.asarray(o["o_pos"]).astype(np.int64)     # [L, 8]
        chunk = np.asarray(o["o_chunk"]).astype(np.int64)  # [L, 8]
        sums = np.asarray(o["o_sums"], np.float32)        # [L, 4]
        m_s = -np.asarray(o["o_negmax"], np.float32)      # [L, 4] slab maxima
        M = m_s.max(axis=1)                               # [L] row max (exact)

        # softmax denominator: Z = sum_s S_s * exp(m_s - M)
        Z = (
            (sums.astype(np.float64) * np.exp((m_s - M[:, None]).astype(np.float64)))
            .sum(axis=1)
            .astype(np.float32)
        )

        # map window positions back to global vocab indices
        w = pos // CHUNK
        r = pos % CHUNK
        gidx = np.take_along_axis(chunk, w, axis=1) * CHUNK + r  # [L, 8]

        # top-5 by (value desc, index asc) — jax.lax.top_k tie semantics
        order = np.lexsort((gidx, -vals), axis=1)[:, :TOP_TOKENS]
        t_idx = np.take_along_axis(gidx, order, axis=1)
        t_val = np.take_along_axis(vals, order, axis=1)

        # float32 softmax prob + log, mirroring the reference's op sequence
        e = np.exp((t_val - M[:, None]).astype(np.float32))
        p = (e / Z[:, None]).astype(np.float32)
        top_logp[b] = np.log(p + EPS).astype(np.float32)
        top_idx[b] = t_idx

    # ---- beam recurrence (faithful reference emulation, float32) ----
    scores = np.full((B, BEAM_WIDTH), -np.inf, np.float32)
    scores[:, 0] = 0.0
    seqs = np.zeros((B, BEAM_WIDTH, L), np.int32)
    KT = BEAM_WIDTH * TOP_TOKENS
    for t in range(L):
        lp = top_logp[:, t]                                # [B, 5]
        idx = top_idx[:, t]                                # [B, 5]
        cand = (scores[:, :, None] + lp[:, None, :]).reshape(B, KT)
        sel = np.argsort(-cand, axis=1, kind="stable")[:, :BEAM_WIDTH]
        scores = np.take_along_axis(cand, sel, axis=1)
        beam = sel // TOP_TOKENS
        tokp = sel % TOP_TOKENS
        toks = np.take_along_axis(idx, tokp, axis=1).astype(np.int32)
        seqs = np.take_along_axis(seqs, beam[:, :, None], axis=1)
        seqs[:, :, t] = toks

    tokens = np.ascontiguousarray(seqs.transpose(0, 2, 1))  # [B, L, 3]
    return tokens, scores


def kernel(logits):
    logits = np.asarray(logits, dtype=np.float32)
    assert logits.shape == (B, L, V), logits.shape
    if os.environ.get("BEAM_NO_HW") == "1":
        core_outs = [_device_outputs_numpy(logits[b]) for b in range(B)]
        kernel.last_exec_time_ns = None
    else:
        core_outs, exec_ns = _run_device(
            [logits[b] for b in range(B)],
            trace=os.environ.get("BEAM_TRACE") == "1",
        )
        kernel.last_exec_time_ns = exec_ns
    return _postprocess(core_outs)


# revision 6
# speedup vs baseline: 1.0330x; 1.0330x over previous
"""Beam search (nn_BeamSearch) on Trainium2 — 8 NeuronCores, pure data parallel.

Input : logits float32 [8, 512, 32000]  (batch, length, vocab)
Output: (tokens int32 [8, 512, 3], scores float32 [8, 3])  — same as reference.

Sharding: batch dim across the 8 cores (one batch element per core, no
cross-core communication).

Device kernel (per core, logits shard [512, 32000]):
  For each 128-row group, stream four [128, 8000] slabs from DRAM and compute
    * per-125-element-chunk maxima (segmented DVE reduce)      -> cm [128, 256]
    * fused exp + accumulate per slab on the scalar engine     -> softmax sums
      (bias = -slab_max, so each slab's sum is sum(exp(x - slab_max)))
  Then per row: top-8 chunks by chunk max (max8 + max_index on cm), gather
  those 8 chunks (1000 elements) with an indirect DMA, and take max8 +
  max_index of the gathered window -> exact top-8 logit values + positions.
  The true top-k elements of a row always live in chunks whose maxima are
  among the top-k chunk maxima, so the window provably contains the row's
  top-8 elements.

Host: rebuild softmax log-probs for the top-5 tokens per (batch, step)
  (log p = (x - row_max) - log Z, with Z assembled from the per-slab sums)
  and run the reference's beam recurrence in float32 over the [8, 512, 5]
  survivors (the heavy 131M-element reduction work all happened on-device).
"""

import os
import numpy as np

B, L, V = 8, 512, 32000
P = 128                 # SBUF partitions (rows per group)
NGROUP = L // P         # 4 row groups per core
SLAB = 4000             # vocab elements per DMA/compute slab
NSLAB = V // SLAB       # 4
CHUNK = 125             # hierarchical top-k chunk size
NCHUNK = V // CHUNK     # 256 chunks per row
CPS = SLAB // CHUNK     # 64 chunks per slab
K8 = 8
BEAM_WIDTH = 3
TOP_TOKENS = 5
EPS = np.float32(2.220446049250313e-16)

_CACHE = {}


def _build_program():
    import concourse.bacc as bacc
    import concourse.tile as tile
    from concourse import bass, mybir

    nc = bacc.Bacc("TRN2", target_bir_lowering=False, debug=False, num_devices=B)

    f32 = mybir.dt.float32
    u32 = mybir.dt.uint32

    lg = nc.dram_tensor("logits", [L, V], f32, kind="ExternalInput")
    o_vals = nc.dram_tensor("o_vals", [L, K8], f32, kind="ExternalOutput")
    o_pos = nc.dram_tensor("o_pos", [L, K8], u32, kind="ExternalOutput")
    o_chunk = nc.dram_tensor("o_chunk", [L, K8], u32, kind="ExternalOutput")
    o_sums = nc.dram_tensor("o_sums", [L, NSLAB], f32, kind="ExternalOutput")
    o_negmax = nc.dram_tensor("o_negmax", [L, NSLAB], f32, kind="ExternalOutput")

    lg_ap = lg.ap()
    # [V-flattened] chunk table view for the indirect gather: row-chunk c of
    # row r is 125 contiguous floats at chunk-id r*256 + c.
    lg_chunks = lg.ap().rearrange("a (c k) -> (a c) k", k=CHUNK)

    with tile.TileContext(nc) as tc:
        with (
            tc.tile_pool(name="slabs", bufs=8) as slab_pool,
            tc.tile_pool(name="trash", bufs=1) as trash_pool,
            tc.tile_pool(name="stats", bufs=3) as stats_pool,
            tc.tile_pool(name="tiny", bufs=3) as tiny_pool,
            tc.tile_pool(name="win", bufs=3) as win_pool,
        ):
            trash = trash_pool.tile([P, SLAB], f32, tag="trash")
            for g in range(NGROUP):
                rows = slice(g * P, (g + 1) * P)
                cm = stats_pool.tile([P, NCHUNK], f32, tag="cm")
                negmax = stats_pool.tile([P, NSLAB], f32, tag="negmax")
                sums = stats_pool.tile([P, NSLAB], f32, tag="sums")
                for s in range(NSLAB):
                    slab = slab_pool.tile([P, SLAB], f32, tag="slab")
                    nc.sync.dma_start(
                        out=slab[:],
                        in_=lg_ap[rows, s * SLAB : (s + 1) * SLAB],
                    )
                    # per-chunk maxima for this slab
                    nc.vector.tensor_reduce(
                        out=cm[:, s * CPS : (s + 1) * CPS],
                        in_=slab[:].rearrange("p (c k) -> p c k", k=CHUNK),
                        axis=mybir.AxisListType.X,
                        op=mybir.AluOpType.max,
                    )
                    # slab max (negated, for the activation bias)
                    nc.vector.tensor_reduce(
                        out=negmax[:, s : s + 1],
                        in_=cm[:, s * CPS : (s + 1) * CPS],
                        axis=mybir.AxisListType.X,
                        op=mybir.AluOpType.max,
                        negate=True,
                    )
                    # sum(exp(x - slab_max)) fused on the scalar engine
                    nc.scalar.activation(
                        out=trash[:],
                        in_=slab[:],
                        func=mybir.ActivationFunctionType.Exp,
                        bias=negmax[:, s : s + 1],
                        scale=1.0,
                        accum_out=sums[:, s : s + 1],
                    )

                # top-8 chunks per row
                top8v = tiny_pool.tile([P, K8], f32, tag="top8v")
                top8c = tiny_pool.tile([P, K8], u32, tag="top8c")
                nc.vector.max(out=top8v[:], in_=cm[:])
                nc.vector.max_index(out=top8c[:], in_max=top8v[:], in_values=cm[:])

                # global chunk ids: (g*128 + p) * 256 + chunk
                gidx = tiny_pool.tile([P, K8], u32, tag="gidx")
                nc.gpsimd.iota(
                    out=gidx[:],
                    pattern=[[0, K8]],
                    base=g * P * NCHUNK,
                    channel_multiplier=NCHUNK,
                )
                nc.vector.tensor_tensor(
                    out=gidx[:], in0=gidx[:], in1=top8c[:], op=mybir.AluOpType.add
                )

                # gather the 8 winning chunks (125 elements each) per row.
                # NB: hardware indirect DMA honors one offset per partition,
                # so issue one gather per window slot.
                win = win_pool.tile([P, K8 * CHUNK], f32, tag="win")
                for j in range(K8):
                    nc.gpsimd.indirect_dma_start(
                        out=win[:, j * CHUNK : (j + 1) * CHUNK],
                        out_offset=None,
                        in_=lg_chunks,
                        in_offset=bass.IndirectOffsetOnAxis(
                            ap=gidx[:, j : j + 1], axis=0
                        ),
                    )

                # exact top-8 of the window = exact top-8 of the row
                wvals = tiny_pool.tile([P, K8], f32, tag="wvals")
                wpos = tiny_pool.tile([P, K8], u32, tag="wpos")
                nc.vector.max(out=wvals[:], in_=win[:])
                nc.vector.max_index(out=wpos[:], in_max=wvals[:], in_values=win[:])

                nc.sync.dma_start(out=o_vals.ap()[rows], in_=wvals[:])
                nc.sync.dma_start(out=o_pos.ap()[rows], in_=wpos[:])
                nc.sync.dma_start(out=o_chunk.ap()[rows], in_=top8c[:])
                nc.sync.dma_start(out=o_sums.ap()[rows], in_=sums[:])
                nc.sync.dma_start(out=o_negmax.ap()[rows], in_=negmax[:])
    nc.compile()
    return nc


def _get_program():
    if "nc" not in _CACHE:
        _CACHE["nc"] = _build_program()
    return _CACHE["nc"]


def _run_device(shards, trace=False):
    """shards: list of 8 [512, 32000] f32 arrays. Returns (per-core outputs,
    exec_time_ns or None)."""
    from concourse.bass_utils import run_bass_kernel_spmd

    nc = _get_program()
    in_maps = [{"logits": np.ascontiguousarray(s)} for s in shards]
    res = run_bass_kernel_spmd(nc, in_maps, core_ids=list(range(len(shards))), trace=trace)
    return res.results, res.exec_time_ns


def _device_outputs_numpy(shard):
    """Bit-faithful numpy emulation of the device kernel for one core.
    Used for validation / fallback (env BEAM_NO_HW=1)."""
    x = shard  # [L, V] f32
    cm = x.reshape(L, NCHUNK, CHUNK).max(axis=2)  # chunk maxima
    slab_max = x.reshape(L, NSLAB, SLAB).max(axis=2)
    negmax = (-slab_max).astype(np.float32)
    ex = np.exp(
        x.reshape(L, NSLAB, SLAB).astype(np.float32) + negmax[:, :, None]
    ).astype(np.float32)
    sums = ex.sum(axis=2, dtype=np.float32)
    # top-8 chunks (by max, ties -> lowest chunk id, descending)
    ordc = np.lexsort((np.arange(NCHUNK)[None, :].repeat(L, 0), -cm), axis=1)[:, :K8]
    top8c = ordc.astype(np.uint32)
    win = np.take_along_axis(
        x.reshape(L, NCHUNK, CHUNK), ordc[:, :, None], axis=1
    ).reshape(L, K8 * CHUNK)
    # device max_index returns positions of the 8 largest values in
    # descending-value order (ties -> successive lowest positions)
    wsort = np.lexsort((np.arange(K8 * CHUNK)[None, :].repeat(L, 0), -win), axis=1)[
        :, :K8
    ]
    wpos = wsort.astype(np.uint32)
    wvals = np.take_along_axis(win, wsort, axis=1).astype(np.float32)
    return {
        "o_vals": wvals,
        "o_pos": wpos,
        "o_chunk": top8c,
        "o_sums": sums,
        "o_negmax": negmax,
    }


def _postprocess(core_outs):
    """core_outs: list of 8 dicts with o_vals/o_pos/o_chunk/o_sums/o_negmax.
    Returns (tokens [8, 512, 3] int32, scores [8, 3] f32), exactly emulating
    the reference's float32 beam recurrence."""
    top_idx = np.empty((B, L, TOP_TOKENS), np.int64)
    top_logp = np.empty((B, L, TOP_TOKENS), np.float32)

    for b, o in enumerate(core_outs):
        vals = np.asarray(o["o_vals"], np.float32)        # [L, 8]
        pos = np.asarray(o["o_pos"]).astype(np.int64)     # [L, 8]
        chunk = np.asarray(o["o_chunk"]).astype(np.int64)  # [L, 8]
        sums = np.asarray(o["o_sums"], np.float32)        # [L, 4]
        m_s = -np.asarray(o["o_negmax"], np.float32)      # [L, 4] slab maxima
        M = m_s.max(axis=1)                               # [L] row max (exact)

        # softmax denominator: Z = sum_s S_s * exp(m_s - M)
        Z = (
            (sums.astype(np.float64) * np.exp((m_s - M[:, None]).astype(np.float64)))
            .sum(axis=1)
            .astype(np.float32)
        )

        # map window positions back to global vocab indices
        w = pos // CHUNK
        r = pos % CHUNK
        gidx = np.take_along_axis(chunk, w, axis=1) * CHUNK + r  # [L, 8]

        # top-5 by (value desc, index asc) — jax.lax.top_k tie semantics
        order = np.lexsort((gidx, -vals), axis=1)[:, :TOP_TOKENS]
        t_idx = np.take_along_axis(gidx, order, axis=1)
        t_val = np.take_along_axis(vals, order, axis=1)

        # float32 softmax prob + log, mirroring the reference's op sequence
        e = np.exp((t_val - M[:, None]).astype(np.float32))
        p = (e / Z[:, None]).astype(np.float32)
        top_logp[b] = np.log(p + EPS).astype(np.float32)
        top_idx[b] = t_idx

    # ---- beam recurrence (faithful reference emulation, float32) ----
    scores = np.full((B, BEAM_WIDTH), -np.inf, np.float32)
    scores[:, 0] = 0.0
    seqs = np.zeros((B, BEAM_WIDTH, L), np.int32)
    KT = BEAM_WIDTH * TOP_TOKENS
    for t in range(L):
        lp = top_logp[:, t]                                # [B, 5]
        idx = top_idx[:, t]                                # [B, 5]
        cand = (scores[:, :, None] + lp[:, None, :]).reshape(B, KT)
        sel = np.argsort(-cand, axis=1, kind="stable")[:, :BEAM_WIDTH]
        scores = np.take_along_axis(cand, sel, axis=1)
        beam = sel // TOP_TOKENS
        tokp = sel % TOP_TOKENS
        toks = np.take_along_axis(idx, tokp, axis=1).astype(np.int32)
        seqs = np.take_along_axis(seqs, beam[:, :, None], axis=1)
        seqs[:, :, t] = toks

    tokens = np.ascontiguousarray(seqs.transpose(0, 2, 1))  # [B, L, 3]
    return tokens, scores


def kernel(logits):
    logits = np.asarray(logits, dtype=np.float32)
    assert logits.shape == (B, L, V), logits.shape
    if os.environ.get("BEAM_NO_HW") == "1":
        core_outs = [_device_outputs_numpy(logits[b]) for b in range(B)]
        kernel.last_exec_time_ns = None
    else:
        core_outs, exec_ns = _run_device(
            [logits[b] for b in range(B)],
            trace=os.environ.get("BEAM_TRACE") == "1",
        )
        kernel.last_exec_time_ns = exec_ns
    return _postprocess(core_outs)
